# revision 24
# baseline (speedup 1.0000x reference)
"""CombinedPeakLoss Trainium2 kernel.

Full inputs y_pred/y_true [16384, 720] f32 -> scalar loss.
Data-parallel over 8 NeuronCores: each core reduces its 2048-row batch
shard to per-partition partial sums [128, 8]; the host combines.

Layout: partitions = batch rows, free dim = 30 days x 24 hours. true/pred
are packed as a [128, 2, 30, 24] pair tile so symmetric ops run once.
Day-grouped reductions are axis-X tensor_reduce on [128, G, k] views;
per-day broadcasts are stride-0 APs. Precision: softmax/argmax compares in
f32; products, window math and squared-error diffs in bf16 (all reduce
accumulation in f32), which keeps total relative error ~1e-4.
Engine split: DVE = reductions + 2-input bf16 ops, ACT = exp/square(+row
accum), GPSIMD = casts + f32 compare + overall-diff.
"""

import sys

sys.path.insert(0, "/opt/trn_rl_repo")

import numpy as np

N_CORES = 8
B_FULL, S = 16384, 720
B = B_FULL // N_CORES          # 2048 rows per core
D, H = 30, 24                  # days, hours
P = 128                        # SBUF partitions
NT = B // P                    # 16 row-tiles per core
NSTAT = 8                      # partial-sum columns (5 used)

# partial-sum columns: sse_all, sse_day, (2*sse_pv + sse_pt), shp_sum, vcnt
C_SSE_ALL, C_SSE_DAY, C_PVPT, C_SHP, C_VCNT = range(5)

_CACHE = {}


def _build(reps=1):
    import contextlib

    import concourse.bacc as bacc
    import concourse.bass as bass
    import concourse.tile as tile
    from concourse import mybir

    f32 = mybir.dt.float32
    bf16 = mybir.dt.bfloat16
    i32 = mybir.dt.int32
    Alu = mybir.AluOpType
    Act = mybir.ActivationFunctionType
    AX = mybir.AxisListType

    def midb(ap, n):
        # insert a stride-0 broadcast dim of size n right after partitions
        l = [list(x) for x in ap.ap]
        l.insert(1, [0, n])
        return bass.AP(tensor=ap.tensor, offset=ap.offset, ap=l)

    nc = bacc.Bacc("TRN2", target_bir_lowering=False, debug=False)
    yt_d = nc.dram_tensor("y_true", [B, S], f32, kind="ExternalInput")
    yp_d = nc.dram_tensor("y_pred", [B, S], f32, kind="ExternalInput")
    out_d = nc.dram_tensor("partials", [P, NSTAT], f32, kind="ExternalOutput")

    with tile.TileContext(nc) as tc:
        with (
            tc.tile_pool(name="consts", bufs=1) as cpool,
            tc.tile_pool(name="io", bufs=4) as io,
            tc.tile_pool(name="work", bufs=3) as wk,
            tc.tile_pool(name="stats", bufs=4) as st,
            tc.tile_pool(name="accp", bufs=1) as accp,
        ):
            # ---- constants (one-time) ----
            # centered hours (h-13 for h in 6..20) duplicated over the t/p
            # pair axis, for e*h'. Only peak-time *differences* matter, so
            # the 13 cancels; centering shrinks bf16 product error ~3x.
            hrs_i = cpool.tile([P, 2, D, 15], i32)
            nc.gpsimd.iota(hrs_i[:], pattern=[[0, 2], [0, D], [1, 15]],
                           base=-7, channel_multiplier=0)
            hoursb = cpool.tile([P, 2, D, 15], bf16)
            nc.vector.tensor_copy(hoursb[:], hrs_i[:])
            # hinv = 24 - h for h in 6..20 (18 down to 4)
            hinv_i = cpool.tile([P, D, 15], i32)
            nc.gpsimd.iota(hinv_i[:], pattern=[[0, D], [-1, 15]],
                           base=18, channel_multiplier=0)
            hinvb = cpool.tile([P, D, 15], bf16)
            nc.vector.tensor_copy(hinvb[:], hinv_i[:])
            # hm24 = h - 24 for h in 4..22 (-20..-2)
            hm24_i = cpool.tile([P, D, 19], i32)
            nc.gpsimd.iota(hm24_i[:], pattern=[[0, D], [1, 19]],
                           base=-20, channel_multiplier=0)
            hm24b = cpool.tile([P, D, 19], bf16)
            nc.vector.tensor_copy(hm24b[:], hm24_i[:])

            acc = accp.tile([P, NSTAT], f32)
            nc.vector.memset(acc[:], 0.0)

            loop_cm = (tc.For_i(0, reps, 1) if reps > 1
                       else contextlib.nullcontext())
            with loop_cm:
              for i in range(NT):
                rows = slice(i * P, (i + 1) * P)
                # [128, pair(t=0,p=1), 30, 24]
                yq = io.tile([P, 2, D, H], f32, tag="yq")
                nc.sync.dma_start(
                    out=yq[:, 0], in_=yt_d[rows, :].rearrange(
                        "p (d h) -> p d h", h=H))
                nc.sync.dma_start(
                    out=yq[:, 1], in_=yp_d[rows, :].rearrange(
                        "p (d h) -> p d h", h=H))
                yb = io.tile([P, 2, D, H], bf16, tag="yb")
                nc.scalar.copy(yb[:], yq[:])

                # ---- soft peaks: night attn is exactly 0 -> day slice only
                # eprod[:,0]=e  [:,1]=e*y  [:,2]=e*h   (bf16, f32 reduce)
                eprod = wk.tile([P, 3, 2, D, 15], bf16, tag="eprod")
                nc.scalar.activation(eprod[:, 0], yq[:, :, :, 6:21],
                                     Act.Exp, scale=10.0)
                nc.vector.tensor_mul(eprod[:, 1], eprod[:, 0],
                                     yb[:, :, :, 6:21])
                nc.gpsimd.tensor_tensor(eprod[:, 2], eprod[:, 0], hoursb[:],
                                        Alu.mult)
                smx = st.tile([P, 3, 2, D], f32, tag="smx")
                nc.vector.reduce_sum(smx[:], eprod[:], axis=AX.X)

                rp = st.tile([P, 2, D], f32, tag="rp")
                nc.vector.reciprocal(rp[:], smx[:, 0])
                # ppk[:,0]=peak_val pair, ppk[:,1]=peak_time pair
                ppk = st.tile([P, 2, 2, D], f32, tag="ppk")
                nc.vector.tensor_mul(ppk[:], smx[:, 1:3], midb(rp[:], 2))
                # fold weights: sqrt(2)*pv so one accum gives 2*Lpv + Lpt
                nc.vector.tensor_scalar_mul(ppk[:, 0], ppk[:, 0],
                                            1.4142135623730951)
                dd = st.tile([P, 2, D], f32, tag="dd")
                nc.vector.tensor_sub(dd[:], ppk[:, :, 1], ppk[:, :, 0])
                ddsq = st.tile([P, 2, D], f32, tag="ddsq")
                a_pp = st.tile([P, 1], f32, tag="a_pp")
                nc.scalar.activation(ddsq[:], dd[:], Act.Square,
                                     accum_out=a_pp[:])

                # ---- argmax of daytime truth (first-index tie-break) ----
                m = st.tile([P, D], f32, tag="m")
                nc.vector.reduce_max(m[:], yq[:, 0, :, 6:21], axis=AX.X)
                eqb = wk.tile([P, D, 15], bf16, tag="eqb")
                nc.vector.tensor_tensor(
                    eqb[:], yq[:, 0, :, 6:21],
                    m[:, :, None].to_broadcast((P, D, 15)), Alu.is_ge)
                qc = wk.tile([P, D, 15], bf16, tag="qc")
                nc.gpsimd.tensor_tensor(qc[:], eqb[:], hinvb[:], Alu.mult)
                q = st.tile([P, D], f32, tag="q")   # 24 - argmax_h
                nc.vector.reduce_max(q[:], qc[:], axis=AX.X)

                # ---- +-2h window on hours 4..22 ----
                w = wk.tile([P, D, 19], bf16, tag="w")      # h - idx
                nc.vector.tensor_tensor(
                    w[:], hm24b[:], q[:, :, None].to_broadcast((P, D, 19)),
                    Alu.add)
                aw = wk.tile([P, D, 19], bf16, tag="aw")
                nc.scalar.activation(aw[:], w[:], Act.Abs)
                w01 = wk.tile([P, D, 19], bf16, tag="w01")
                nc.vector.tensor_scalar(w01[:], aw[:], 2.0, None, Alu.is_le)

                yw = wk.tile([P, 2, D, 19], bf16, tag="yw")
                nc.vector.tensor_mul(yw[:], yb[:, :, :, 4:23],
                                     midb(w01[:], 2))
                tpm = st.tile([P, 2, D], f32, tag="tpm")
                nc.vector.reduce_max(tpm[:], yw[:], axis=AX.X)

                vv = st.tile([P, 2, D], f32, tag="vv")      # [vm, valid]
                nc.vector.tensor_scalar(vv[:, 1], tpm[:, 0], 1e-6, None,
                                        Alu.is_gt)
                pme = st.tile([P, 2, D], f32, tag="pme")
                # tmax_safe: exact when valid; keeps invalid days finite
                nc.vector.tensor_scalar(pme[:, 0], tpm[:, 0], 1e-3, None,
                                        Alu.max)
                nc.vector.tensor_scalar(pme[:, 1], tpm[:, 1], 1e-6, None,
                                        Alu.add)
                r2 = st.tile([P, 2, D], f32, tag="r2")
                nc.vector.reciprocal(r2[:], pme[:])
                # replicate over hours on ACT so the multiply keeps 2x mode
                r2r = wk.tile([P, 2, D, 19], bf16, tag="r2r")
                nc.scalar.copy(r2r[:], r2[:].to_broadcast((P, 2, D, 19)))

                ndp = wk.tile([P, 2, D, 19], bf16, tag="ndp")
                nc.vector.tensor_mul(ndp[:], yw[:], r2r[:])
                nd = wk.tile([P, D, 19], bf16, tag="nd")
                nc.gpsimd.tensor_tensor(nd[:], ndp[:, 1], ndp[:, 0],
                                        Alu.subtract)
                nds = wk.tile([P, D, 19], f32, tag="nds")
                nc.scalar.activation(nds[:], nd[:], Act.Square)
                msum = st.tile([P, D], f32, tag="msum")
                nc.vector.reduce_sum(msum[:], nds[:], axis=AX.X)
                nc.vector.tensor_mul(vv[:, 0], msum[:], vv[:, 1])
                vsum = st.tile([P, 2], f32, tag="vsum")
                nc.vector.reduce_sum(vsum[:], vv[:], axis=AX.X)

                # ---- overall + daytime SSE (bf16 diff, f32 row accum) ----
                dall = wk.tile([P, D, H], bf16, tag="dall")
                nc.gpsimd.tensor_tensor(dall[:], yb[:, 1], yb[:, 0],
                                        Alu.subtract)
                aa = st.tile([P, 2], f32, tag="aa")
                dsq = wk.tile([P, D, H], f32, tag="dsq")
                nc.scalar.activation(dsq[:], dall[:], Act.Square,
                                     accum_out=aa[:, 0:1])
                dsqd = wk.tile([P, D, 15], f32, tag="dsqd")
                nc.scalar.activation(dsqd[:], dall[:, :, 6:21], Act.Square,
                                     accum_out=aa[:, 1:2])

                # ---- accumulate ----
                nc.vector.tensor_add(acc[:, 0:2], acc[:, 0:2], aa[:])
                nc.vector.tensor_add(acc[:, 2:3], acc[:, 2:3], a_pp[:])
                nc.vector.tensor_add(acc[:, 3:5], acc[:, 3:5], vsum[:])

            nc.sync.dma_start(out=out_d[:], in_=acc[:])

    nc.compile()
    return nc


def _get_nc(reps=1):
    key = ("nc", reps)
    if key not in _CACHE:
        _CACHE[key] = _build(reps)
    return _CACHE[key]


def _combine(partials_stack):
    tot = partials_stack.astype(np.float64).reshape(-1, NSTAT).sum(axis=0)
    n_all = float(B_FULL * S)
    n_days = float(B_FULL * D)
    total = (tot[C_SSE_ALL] / n_all                    # L_overall
             + 0.5 * tot[C_SSE_DAY] / n_all            # 0.5 * L_daytime
             + tot[C_PVPT] / n_days                    # 2*L_pv + L_pt
             + 0.5 * (tot[C_SHP] / 5.0) / (tot[C_VCNT] + 1e-6))
    return np.float32(total)


def _run(y_pred, y_true, trace=False):
    from concourse.bass_utils import run_bass_kernel_spmd

    y_pred = np.ascontiguousarray(np.asarray(y_pred, dtype=np.float32))
    y_true = np.ascontiguousarray(np.asarray(y_true, dtype=np.float32))
    assert y_pred.shape == (B_FULL, S) and y_true.shape == (B_FULL, S)

    nc = _get_nc()
    in_maps = [
        {"y_true": y_true[c * B:(c + 1) * B],
         "y_pred": y_pred[c * B:(c + 1) * B]}
        for c in range(N_CORES)
    ]
    res = run_bass_kernel_spmd(nc, in_maps, list(range(N_CORES)), trace=trace)
    parts = np.stack([res.results[c]["partials"] for c in range(N_CORES)])
    return _combine(parts), res.exec_time_ns


def kernel(**inputs):
    out, _ = _run(inputs["y_pred"], inputs["y_true"])
    return out


def _make_runner(nc):
    """Persistent jitted executable + device-resident inputs, mirroring
    bass2jax.run_bass_via_pjrt but reusable across calls for timing."""
    import jax
    from jax.experimental.shard_map import shard_map
    from jax.sharding import Mesh, NamedSharding, PartitionSpec

    from concourse import bass2jax, mybir

    bass2jax.install_neuronx_cc_hook()

    in_names, out_names, out_avals = [], [], []
    part_name = nc.partition_id_tensor.name if nc.partition_id_tensor else None
    for alloc in nc.m.functions[0].allocations:
        if not isinstance(alloc, mybir.MemoryLocationSet):
            continue
        name = alloc.memorylocations[0].name
        if alloc.kind == "ExternalInput":
            if name != part_name:
                in_names.append(name)
        elif alloc.kind == "ExternalOutput":
            out_names.append(name)
            out_avals.append(jax.core.ShapedArray(
                tuple(alloc.tensor_shape), mybir.dt.np(alloc.dtype)))
    assert nc.dbg_addr is None
    n_params = len(in_names)
    n_outs = len(out_names)
    all_in_names = in_names + out_names
    if part_name is not None:
        all_in_names = all_in_names + [part_name]
    donate = tuple(range(n_params, n_params + n_outs))

    def _body(*args):
        operands = list(args)
        if part_name is not None:
            operands.append(bass2jax.partition_id_tensor())
        outs = bass2jax._bass_exec_p.bind(
            *operands,
            out_avals=tuple(out_avals),
            in_names=tuple(all_in_names),
            out_names=tuple(out_names),
            lowering_input_output_aliases=(),
            sim_require_finite=True,
            sim_require_nnan=True,
            nc=nc,
        )
        return tuple(outs)

    devices = jax.devices()[:N_CORES]
    mesh = Mesh(np.asarray(devices), ("core",))
    pc = PartitionSpec("core")
    fn = jax.jit(
        shard_map(_body, mesh=mesh, in_specs=(pc,) * (n_params + n_outs),
                  out_specs=(pc,) * n_outs, check_rep=False),
        donate_argnums=donate, keep_unused=True)
    sharding = NamedSharding(mesh, pc)

    class Runner:
        def put(self, in_maps):
            import jax as _jax
            self.xin = [
                _jax.device_put(
                    np.concatenate([np.asarray(m[name]) for m in in_maps],
                                   axis=0), sharding)
                for name in in_names
            ]

        def __call__(self):
            import jax as _jax
            zeros = [np.zeros((N_CORES * a.shape[0], *a.shape[1:]), a.dtype)
                     for a in out_avals]
            outs = fn(*self.xin, *zeros)
            outs = _jax.block_until_ready(outs)
            return {
                name: np.asarray(outs[i]).reshape(N_CORES,
                                                  *out_avals[i].shape)
                for i, name in enumerate(out_names)
            }

    return Runner()


def bench(y_pred, y_true, reps_pair=(1, 129), n_calls=8):
    """Per-kernel device time from the wall-clock slope between a 1-rep
    and an R-rep (device-side loop) variant of the same program."""
    import time

    y_pred = np.ascontiguousarray(np.asarray(y_pred, dtype=np.float32))
    y_true = np.ascontiguousarray(np.asarray(y_true, dtype=np.float32))
    in_maps = [
        {"y_true": y_true[c * B:(c + 1) * B],
         "y_pred": y_pred[c * B:(c + 1) * B]}
        for c in range(N_CORES)
    ]
    walls = {}
    out_lo = None
    for reps in reps_pair:
        nc = _get_nc(reps)
        runner = _make_runner(nc)
        runner.put(in_maps)
        runner()  # warmup / compile
        best = float("inf")
        for _ in range(n_calls):
            t0 = time.perf_counter()
            out = runner()
            best = min(best, time.perf_counter() - t0)
        walls[reps] = best
        if reps == reps_pair[0]:
            out_lo = out
    r0, r1 = reps_pair
    per_rep_ns = (walls[r1] - walls[r0]) / (r1 - r0) * 1e9
    return per_rep_ns, walls, out_lo


# revision 28
# speedup vs baseline: 1.0111x; 1.0111x over previous
"""CombinedPeakLoss Trainium2 kernel.

Full inputs y_pred/y_true [16384, 720] f32 -> scalar loss.
Data-parallel over 8 NeuronCores: each core reduces its 2048-row batch
shard to per-partition partial sums [128, 8]; the host combines.

Layout: partitions = batch rows, free dim = 30 days x 24 hours. true/pred
are packed as a [128, 2, 30, 24] pair tile so symmetric ops run once.
Day-grouped reductions are axis-X tensor_reduce on [128, G, k] views;
per-day broadcasts are stride-0 APs. Precision: softmax/argmax compares in
f32; products, window math and squared-error diffs in bf16 (all reduce
accumulation in f32), which keeps total relative error ~1e-4.
Engine split: DVE = reductions + 2-input bf16 ops, ACT = exp/square(+row
accum), GPSIMD = casts + f32 compare + overall-diff.
"""

import sys

sys.path.insert(0, "/opt/trn_rl_repo")

import numpy as np

N_CORES = 8
B_FULL, S = 16384, 720
B = B_FULL // N_CORES          # 2048 rows per core
D, H = 30, 24                  # days, hours
P = 128                        # SBUF partitions
NT = B // P                    # 16 row-tiles per core
NSTAT = 8                      # partial-sum columns (5 used)

# partial-sum columns: sse_all, sse_day, (2*sse_pv + sse_pt), shp_sum, vcnt
C_SSE_ALL, C_SSE_DAY, C_PVPT, C_SHP, C_VCNT = range(5)

_CACHE = {}


def _build(reps=1):
    import contextlib

    import concourse.bacc as bacc
    import concourse.bass as bass
    import concourse.tile as tile
    from concourse import mybir

    f32 = mybir.dt.float32
    bf16 = mybir.dt.bfloat16
    i32 = mybir.dt.int32
    Alu = mybir.AluOpType
    Act = mybir.ActivationFunctionType
    AX = mybir.AxisListType

    def midb(ap, n):
        # insert a stride-0 broadcast dim of size n right after partitions
        l = [list(x) for x in ap.ap]
        l.insert(1, [0, n])
        return bass.AP(tensor=ap.tensor, offset=ap.offset, ap=l)

    nc = bacc.Bacc("TRN2", target_bir_lowering=False, debug=False)
    yt_d = nc.dram_tensor("y_true", [B, S], f32, kind="ExternalInput")
    yp_d = nc.dram_tensor("y_pred", [B, S], f32, kind="ExternalInput")
    out_d = nc.dram_tensor("partials", [P, NSTAT], f32, kind="ExternalOutput")

    with tile.TileContext(nc) as tc:
        with (
            tc.tile_pool(name="consts", bufs=1) as cpool,
            tc.tile_pool(name="io", bufs=4) as io,
            tc.tile_pool(name="work", bufs=3) as wk,
            tc.tile_pool(name="stats", bufs=4) as st,
            tc.tile_pool(name="accp", bufs=1) as accp,
        ):
            # ---- constants (one-time) ----
            # centered hours (h-13 for h in 6..20) duplicated over the t/p
            # pair axis, for e*h'. Only peak-time *differences* matter, so
            # the 13 cancels; centering shrinks bf16 product error ~3x.
            hrs_i = cpool.tile([P, 2, D, 15], i32)
            nc.gpsimd.iota(hrs_i[:], pattern=[[0, 2], [0, D], [1, 15]],
                           base=-7, channel_multiplier=0)
            hoursb = cpool.tile([P, 2, D, 15], bf16)
            nc.vector.tensor_copy(hoursb[:], hrs_i[:])
            # hinv = 24 - h for h in 6..20 (18 down to 4)
            hinv_i = cpool.tile([P, D, 15], i32)
            nc.gpsimd.iota(hinv_i[:], pattern=[[0, D], [-1, 15]],
                           base=18, channel_multiplier=0)
            hinvb = cpool.tile([P, D, 15], bf16)
            nc.vector.tensor_copy(hinvb[:], hinv_i[:])
            # hm24 = h - 24 for h in 4..22 (-20..-2)
            hm24_i = cpool.tile([P, D, 19], i32)
            nc.gpsimd.iota(hm24_i[:], pattern=[[0, D], [1, 19]],
                           base=-20, channel_multiplier=0)
            hm24b = cpool.tile([P, D, 19], bf16)
            nc.vector.tensor_copy(hm24b[:], hm24_i[:])

            acc = accp.tile([P, NSTAT], f32)
            nc.vector.memset(acc[:], 0.0)

            loop_cm = (tc.For_i(0, reps, 1) if reps > 1
                       else contextlib.nullcontext())
            with loop_cm:
              for i in range(NT):
                rows = slice(i * P, (i + 1) * P)
                # [128, pair(t=0,p=1), 30, 24]
                yq = io.tile([P, 2, D, H], f32, tag="yq")
                nc.sync.dma_start(
                    out=yq[:, 0], in_=yt_d[rows, :].rearrange(
                        "p (d h) -> p d h", h=H))
                nc.sync.dma_start(
                    out=yq[:, 1], in_=yp_d[rows, :].rearrange(
                        "p (d h) -> p d h", h=H))
                yb = io.tile([P, 2, D, H], bf16, tag="yb")
                nc.scalar.copy(yb[:], yq[:])

                # ---- soft peaks: night attn is exactly 0 -> day slice only
                # eprod[:,0]=e  [:,1]=e*y  [:,2]=e*h   (bf16, f32 reduce)
                eprod = wk.tile([P, 3, 2, D, 15], bf16, tag="eprod")
                nc.scalar.activation(eprod[:, 0], yq[:, :, :, 6:21],
                                     Act.Exp, scale=10.0)
                nc.vector.tensor_mul(eprod[:, 1], eprod[:, 0],
                                     yb[:, :, :, 6:21])
                nc.gpsimd.tensor_tensor(eprod[:, 2], eprod[:, 0], hoursb[:],
                                        Alu.mult)
                smx = st.tile([P, 3, 2, D], f32, tag="smx")
                nc.vector.reduce_sum(smx[:], eprod[:], axis=AX.X)

                rp = st.tile([P, 2, D], f32, tag="rp")
                nc.vector.reciprocal(rp[:], smx[:, 0])
                # ppk[:,0]=peak_val pair, ppk[:,1]=peak_time pair
                ppk = st.tile([P, 2, 2, D], f32, tag="ppk")
                nc.vector.tensor_mul(ppk[:], smx[:, 1:3], midb(rp[:], 2))
                # fold weights: sqrt(2)*pv so one accum gives 2*Lpv + Lpt
                nc.vector.tensor_scalar_mul(ppk[:, 0], ppk[:, 0],
                                            1.4142135623730951)
                dd = st.tile([P, 2, D], f32, tag="dd")
                nc.vector.tensor_sub(dd[:], ppk[:, :, 1], ppk[:, :, 0])
                ddsq = st.tile([P, 2, D], f32, tag="ddsq")
                a_pp = st.tile([P, 1], f32, tag="a_pp")
                nc.scalar.activation(ddsq[:], dd[:], Act.Square,
                                     accum_out=a_pp[:])

                # ---- argmax of daytime truth (first-index tie-break) ----
                m = st.tile([P, D], f32, tag="m")
                nc.vector.reduce_max(m[:], yq[:, 0, :, 6:21], axis=AX.X)
                eqb = wk.tile([P, D, 15], bf16, tag="eqb")
                nc.vector.tensor_tensor(
                    eqb[:], yq[:, 0, :, 6:21],
                    m[:, :, None].to_broadcast((P, D, 15)), Alu.is_ge)
                qc = wk.tile([P, D, 15], bf16, tag="qc")
                nc.gpsimd.tensor_tensor(qc[:], eqb[:], hinvb[:], Alu.mult)
                q = st.tile([P, D], f32, tag="q")   # 24 - argmax_h
                nc.vector.reduce_max(q[:], qc[:], axis=AX.X)

                # ---- +-2h window on hours 4..22 ----
                # replicate q over hours on ACT so the add keeps 2x mode
                qr = wk.tile([P, D, 19], bf16, tag="qr")
                nc.scalar.copy(qr[:],
                               q[:, :, None].to_broadcast((P, D, 19)))
                w = wk.tile([P, D, 19], bf16, tag="w")      # h - idx
                nc.vector.tensor_tensor(w[:], hm24b[:], qr[:], Alu.add)
                aw = wk.tile([P, D, 19], bf16, tag="aw")
                nc.scalar.activation(aw[:], w[:], Act.Abs)
                w01 = wk.tile([P, D, 19], bf16, tag="w01")
                nc.vector.tensor_scalar(w01[:], aw[:], 2.0, None, Alu.is_le)

                yw = wk.tile([P, 2, D, 19], bf16, tag="yw")
                nc.vector.tensor_mul(yw[:], yb[:, :, :, 4:23],
                                     midb(w01[:], 2))
                tpm = st.tile([P, 2, D], f32, tag="tpm")
                nc.vector.reduce_max(tpm[:], yw[:], axis=AX.X)

                vv = st.tile([P, 2, D], f32, tag="vv")      # [vm, valid]
                nc.vector.tensor_scalar(vv[:, 1], tpm[:, 0], 1e-6, None,
                                        Alu.is_gt)
                pme = st.tile([P, 2, D], f32, tag="pme")
                # tmax_safe: exact when valid; keeps invalid days finite
                nc.vector.tensor_scalar(pme[:, 0], tpm[:, 0], 1e-3, None,
                                        Alu.max)
                nc.vector.tensor_scalar(pme[:, 1], tpm[:, 1], 1e-6, None,
                                        Alu.add)
                r2 = st.tile([P, 2, D], f32, tag="r2")
                nc.vector.reciprocal(r2[:], pme[:])
                # replicate over hours on ACT so the multiply keeps 2x mode
                r2r = wk.tile([P, 2, D, 19], bf16, tag="r2r")
                nc.scalar.copy(r2r[:], r2[:].to_broadcast((P, 2, D, 19)))

                ndp = wk.tile([P, 2, D, 19], bf16, tag="ndp")
                nc.vector.tensor_mul(ndp[:], yw[:], r2r[:])
                nd = wk.tile([P, D, 19], bf16, tag="nd")
                nc.gpsimd.tensor_tensor(nd[:], ndp[:, 1], ndp[:, 0],
                                        Alu.subtract)
                nds = wk.tile([P, D, 19], f32, tag="nds")
                nc.scalar.activation(nds[:], nd[:], Act.Square)
                msum = st.tile([P, D], f32, tag="msum")
                nc.vector.reduce_sum(msum[:], nds[:], axis=AX.X)
                nc.vector.tensor_mul(vv[:, 0], msum[:], vv[:, 1])
                vsum = st.tile([P, 2], f32, tag="vsum")
                nc.vector.reduce_sum(vsum[:], vv[:], axis=AX.X)

                # ---- overall + daytime SSE (bf16 diff, f32 row accum) ----
                dall = wk.tile([P, D, H], bf16, tag="dall")
                nc.gpsimd.tensor_tensor(dall[:], yb[:, 1], yb[:, 0],
                                        Alu.subtract)
                aa = st.tile([P, 2], f32, tag="aa")
                dsq = wk.tile([P, D, H], f32, tag="dsq")
                nc.scalar.activation(dsq[:], dall[:], Act.Square,
                                     accum_out=aa[:, 0:1])
                dsqd = wk.tile([P, D, 15], f32, tag="dsqd")
                nc.scalar.activation(dsqd[:], dall[:, :, 6:21], Act.Square,
                                     accum_out=aa[:, 1:2])

                # ---- accumulate (Pool: keeps the serialized RMW chain off
                # the critical DVE queue) ----
                nc.gpsimd.tensor_tensor(acc[:, 0:2], acc[:, 0:2], aa[:],
                                        Alu.add)
                nc.gpsimd.tensor_tensor(acc[:, 2:3], acc[:, 2:3], a_pp[:],
                                        Alu.add)
                nc.gpsimd.tensor_tensor(acc[:, 3:5], acc[:, 3:5], vsum[:],
                                        Alu.add)

            nc.sync.dma_start(out=out_d[:], in_=acc[:])

    nc.compile()
    return nc


def _get_nc(reps=1):
    key = ("nc", reps)
    if key not in _CACHE:
        _CACHE[key] = _build(reps)
    return _CACHE[key]


def _combine(partials_stack):
    tot = partials_stack.astype(np.float64).reshape(-1, NSTAT).sum(axis=0)
    n_all = float(B_FULL * S)
    n_days = float(B_FULL * D)
    total = (tot[C_SSE_ALL] / n_all                    # L_overall
             + 0.5 * tot[C_SSE_DAY] / n_all            # 0.5 * L_daytime
             + tot[C_PVPT] / n_days                    # 2*L_pv + L_pt
             + 0.5 * (tot[C_SHP] / 5.0) / (tot[C_VCNT] + 1e-6))
    return np.float32(total)


def _run(y_pred, y_true, trace=False):
    from concourse.bass_utils import run_bass_kernel_spmd

    y_pred = np.ascontiguousarray(np.asarray(y_pred, dtype=np.float32))
    y_true = np.ascontiguousarray(np.asarray(y_true, dtype=np.float32))
    assert y_pred.shape == (B_FULL, S) and y_true.shape == (B_FULL, S)

    nc = _get_nc()
    in_maps = [
        {"y_true": y_true[c * B:(c + 1) * B],
         "y_pred": y_pred[c * B:(c + 1) * B]}
        for c in range(N_CORES)
    ]
    res = run_bass_kernel_spmd(nc, in_maps, list(range(N_CORES)), trace=trace)
    parts = np.stack([res.results[c]["partials"] for c in range(N_CORES)])
    return _combine(parts), res.exec_time_ns


def kernel(**inputs):
    out, _ = _run(inputs["y_pred"], inputs["y_true"])
    return out


def _make_runner(nc):
    """Persistent jitted executable + device-resident inputs, mirroring
    bass2jax.run_bass_via_pjrt but reusable across calls for timing."""
    import jax
    from jax.experimental.shard_map import shard_map
    from jax.sharding import Mesh, NamedSharding, PartitionSpec

    from concourse import bass2jax, mybir

    bass2jax.install_neuronx_cc_hook()

    in_names, out_names, out_avals = [], [], []
    part_name = nc.partition_id_tensor.name if nc.partition_id_tensor else None
    for alloc in nc.m.functions[0].allocations:
        if not isinstance(alloc, mybir.MemoryLocationSet):
            continue
        name = alloc.memorylocations[0].name
        if alloc.kind == "ExternalInput":
            if name != part_name:
                in_names.append(name)
        elif alloc.kind == "ExternalOutput":
            out_names.append(name)
            out_avals.append(jax.core.ShapedArray(
                tuple(alloc.tensor_shape), mybir.dt.np(alloc.dtype)))
    assert nc.dbg_addr is None
    n_params = len(in_names)
    n_outs = len(out_names)
    all_in_names = in_names + out_names
    if part_name is not None:
        all_in_names = all_in_names + [part_name]
    donate = tuple(range(n_params, n_params + n_outs))

    def _body(*args):
        operands = list(args)
        if part_name is not None:
            operands.append(bass2jax.partition_id_tensor())
        outs = bass2jax._bass_exec_p.bind(
            *operands,
            out_avals=tuple(out_avals),
            in_names=tuple(all_in_names),
            out_names=tuple(out_names),
            lowering_input_output_aliases=(),
            sim_require_finite=True,
            sim_require_nnan=True,
            nc=nc,
        )
        return tuple(outs)

    devices = jax.devices()[:N_CORES]
    mesh = Mesh(np.asarray(devices), ("core",))
    pc = PartitionSpec("core")
    fn = jax.jit(
        shard_map(_body, mesh=mesh, in_specs=(pc,) * (n_params + n_outs),
                  out_specs=(pc,) * n_outs, check_rep=False),
        donate_argnums=donate, keep_unused=True)
    sharding = NamedSharding(mesh, pc)

    class Runner:
        def put(self, in_maps):
            import jax as _jax
            self.xin = [
                _jax.device_put(
                    np.concatenate([np.asarray(m[name]) for m in in_maps],
                                   axis=0), sharding)
                for name in in_names
            ]

        def __call__(self):
            import jax as _jax
            zeros = [np.zeros((N_CORES * a.shape[0], *a.shape[1:]), a.dtype)
                     for a in out_avals]
            outs = fn(*self.xin, *zeros)
            outs = _jax.block_until_ready(outs)
            return {
                name: np.asarray(outs[i]).reshape(N_CORES,
                                                  *out_avals[i].shape)
                for i, name in enumerate(out_names)
            }

    return Runner()


def bench(y_pred, y_true, reps_pair=(1, 129), n_calls=8):
    """Per-kernel device time from the wall-clock slope between a 1-rep
    and an R-rep (device-side loop) variant of the same program."""
    import time

    y_pred = np.ascontiguousarray(np.asarray(y_pred, dtype=np.float32))
    y_true = np.ascontiguousarray(np.asarray(y_true, dtype=np.float32))
    in_maps = [
        {"y_true": y_true[c * B:(c + 1) * B],
         "y_pred": y_pred[c * B:(c + 1) * B]}
        for c in range(N_CORES)
    ]
    walls = {}
    out_lo = None
    for reps in reps_pair:
        nc = _get_nc(reps)
        runner = _make_runner(nc)
        runner.put(in_maps)
        runner()  # warmup / compile
        best = float("inf")
        for _ in range(n_calls):
            t0 = time.perf_counter()
            out = runner()
            best = min(best, time.perf_counter() - t0)
        walls[reps] = best
        if reps == reps_pair[0]:
            out_lo = out
    r0, r1 = reps_pair
    per_rep_ns = (walls[r1] - walls[r0]) / (r1 - r0) * 1e9
    return per_rep_ns, walls, out_lo
